# revision 1
# baseline (speedup 1.0000x reference)
"""BiaffineSpan TRN2 kernel.

Full-input contract: kernel(**inputs) -> [B, L, L, C] float32.

Sharding: the C=256 bilinear channel dim is split across 8 NeuronCores
(32 channels each).  Each core computes, entirely on-device:

    Hs = MLP_s(hidden)            # [B, L, D]   (dup on every core)
    He = MLP_e(hidden)            # [B, L, D]
    T[c]  = Hs[b] @ W1[c]         # stage 1, per local channel
    S[c]  = T[c] @ He[b].T        # stage 2
    S[c] += Ls[b,i,c] + Le[b,j,c] + W2_b[c] + bias[c]

All matrices are kept transposed ([feature, token]) on-chip so both
matmul stages contract over the partition dim with no on-device
transposes.  Host side only reshapes/casts inputs and concatenates the
8 per-core [B, 32, L, L] outputs.

Numerics mode (matmul input dtype) via env BIAFFINE_MODE; the default
"mix" runs the MLPs in fp32r (TF32-like single-pass PE mode) and the two
big bilinear stages in bf16 with fp32 PSUM accumulation — measured
5.5e-3 -> 3.3e-3 max-abs/absmax error vs the fp32 reference for ~1% time.
"""

import os
from contextlib import ExitStack

import numpy as np
import ml_dtypes

import concourse.bass as bass
import concourse.bacc as bacc
import concourse.mybir as mybir
import concourse.tile as tile
from concourse.bass_utils import run_bass_kernel_spmd

B, L, D, C = 2, 512, 768, 256
NCORES = 8
CLOC = C // NCORES          # 32 channels per core
T = B * L                   # 1024 tokens
P = 128
DT = D // P                 # 6 feature tiles
LT = L // P                 # 4 token tiles per batch el
TT = T // P                 # 8 token tiles total
NCH = T // 512              # 2 moving chunks of 512 tokens

F32 = mybir.dt.float32

MODE = os.environ.get("BIAFFINE_MODE", "mix")
# Matmul-operand dtypes per phase: (mlp, stage1, stage2).
# fp32r: fp32 storage, single-pass PE matmul with rounded multiply (TF32-ish).
# The BIR verifier requires every producer feeding an fp32r matmul to emit
# fp32r-tagged outputs, so operand tiles carry the dtype end-to-end
# (numpy side is plain float32).
_BF, _FR, _FF = mybir.dt.bfloat16, mybir.dt.float32r, mybir.dt.float32
_MODES = {
    "bf16": (_BF, _BF, _BF),
    "f32r": (_FR, _FR, _FR),
    "f32": (_FF, _FF, _FF),
    "mix": (_FR, _BF, _BF),    # accurate MLP, fast stages
    "mix2": (_FR, _BF, _FR),   # + accurate stage-2
}
DT_MLP, DT_S1, DT_S2 = _MODES[MODE]


def _np_of(dt):
    return ml_dtypes.bfloat16 if dt == _BF else np.float32


def build_program():
    nc = bacc.Bacc("TRN2", target_bir_lowering=False, debug=False)

    # ---- DRAM parameters (per-core inputs) ----
    xT_h = nc.declare_dram_parameter("xT", [D, T], DT_MLP, isOutput=False)
    sw1T_h = nc.declare_dram_parameter("sw1T", [D, D], DT_MLP, isOutput=False)
    sw2T_h = nc.declare_dram_parameter("sw2T", [D, D], DT_MLP, isOutput=False)
    ew1T_h = nc.declare_dram_parameter("ew1T", [D, D], DT_MLP, isOutput=False)
    ew2T_h = nc.declare_dram_parameter("ew2T", [D, D], DT_MLP, isOutput=False)
    sb1_h = nc.declare_dram_parameter("sb1", [D], F32, isOutput=False)
    sb2_h = nc.declare_dram_parameter("sb2", [D], F32, isOutput=False)
    eb1_h = nc.declare_dram_parameter("eb1", [D], F32, isOutput=False)
    eb2_h = nc.declare_dram_parameter("eb2", [D], F32, isOutput=False)
    w1c_h = nc.declare_dram_parameter("w1c", [CLOC, D, D], DT_S1, isOutput=False)
    wsT_h = nc.declare_dram_parameter("wsT", [D, CLOC], DT_S1, isOutput=False)
    weT_h = nc.declare_dram_parameter("weT", [D, CLOC], DT_S2, isOutput=False)
    w0_h = nc.declare_dram_parameter("w0", [CLOC, 1], F32, isOutput=False)
    out_h = nc.declare_dram_parameter("out", [B, CLOC, L, L], F32, isOutput=True)
    # DRAM bounce for Le rows, so they can be partition-broadcast back in
    leD_h = nc.dram_tensor("leD", [CLOC, T], F32)

    Relu = mybir.ActivationFunctionType.Relu
    Ident = mybir.ActivationFunctionType.Identity

    def mm(ps, lhsT, rhs, start, stop):
        nc.tensor.matmul(ps, lhsT, rhs, start=start, stop=stop)

    with tile.TileContext(nc) as tc, ExitStack() as ctx:
        # persistent pools
        p_h = ctx.enter_context(tc.tile_pool(name="hids", bufs=1))
        p_lin = ctx.enter_context(tc.tile_pool(name="lin", bufs=1))

        h1T = p_h.tile([P, DT, T], DT_MLP, tag="h1")
        hsT = p_h.tile([P, DT, T], DT_S1, tag="hs")
        heT = p_h.tile([P, DT, T], DT_S2, tag="he")

        # ---------------- Phase A: the two MLPs ----------------
        with (
            tc.tile_pool(name="ph_a", bufs=2) as p_a,
            tc.tile_pool(name="ps_a", bufs=4, space="PSUM") as ps_a,
            tc.tile_pool(name="bias", bufs=1) as p_bias,
        ):
            xT = p_a.tile([P, DT, T], DT_MLP, tag="x")
            xT_ap = xT_h[:].rearrange("(t p) n -> t p n", p=P)
            for kt in range(DT):
                nc.gpsimd.dma_start(out=xT[:, kt, :], in_=xT_ap[kt])
            b_sb = {}
            for nm, h in (("sb1", sb1_h), ("sb2", sb2_h),
                          ("eb1", eb1_h), ("eb2", eb2_h)):
                b_sb[nm] = p_bias.tile([P, DT], F32, tag=nm, name=nm)
                nc.gpsimd.dma_start(
                    out=b_sb[nm][:], in_=h[:].rearrange("(t p) -> p t", p=P)
                )

            def linear(inT, w_h, bias_t, outT, relu):
                wT = p_a.tile([P, DT, D], DT_MLP, tag="w_mlp")
                w_ap = w_h[:].rearrange("(t p) o -> t p o", p=P)
                for kt in range(DT):
                    nc.gpsimd.dma_start(out=wT[:, kt, :], in_=w_ap[kt])
                for ot in range(DT):
                    for chk in range(NCH):
                        ps = ps_a.tile([P, 512], F32, tag="ps_mlp")
                        for kt in range(DT):
                            mm(ps[:],
                               wT[:, kt, ot * P:(ot + 1) * P],
                               inT[:, kt, chk * 512:(chk + 1) * 512],
                               start=(kt == 0), stop=(kt == DT - 1))
                        nc.scalar.activation(
                            outT[:, ot, chk * 512:(chk + 1) * 512], ps[:],
                            Relu if relu else Ident,
                            bias=bias_t[:, ot:ot + 1])

            linear(xT, sw1T_h, b_sb["sb1"], h1T, relu=True)
            linear(h1T, sw2T_h, b_sb["sb2"], hsT, relu=False)
            linear(xT, ew1T_h, b_sb["eb1"], h1T, relu=True)
            linear(h1T, ew2T_h, b_sb["eb2"], heT, relu=False)

        # ---------------- Phase B: linear terms ----------------
        # LsP [tok_tile, 128, CLOC]  (token on partitions, channel free)
        # LeT [CLOC, T] (+ w0)      (channel on partitions, token free)
        lsP = p_lin.tile([P, TT, CLOC], F32, tag="lsP")
        leT = p_lin.tile([CLOC, T], F32, tag="leT")

        with (
            tc.tile_pool(name="ps_b", bufs=2, space="PSUM") as ps_b,
            tc.tile_pool(name="wse", bufs=1) as p_wse,
        ):
            wsT = p_wse.tile([P, DT, CLOC], DT_S1, tag="ws")
            weT = p_wse.tile([P, DT, CLOC], DT_S2, tag="we")
            w0_sb = p_wse.tile([CLOC, 1], F32, tag="w0")
            nc.gpsimd.dma_start(
                out=wsT[:], in_=wsT_h[:].rearrange("(t p) c -> p t c", p=P))
            nc.gpsimd.dma_start(
                out=weT[:], in_=weT_h[:].rearrange("(t p) c -> p t c", p=P))
            nc.gpsimd.dma_start(out=w0_sb[:], in_=w0_h[:])

            for tt_ in range(TT):
                ps = ps_b.tile([P, CLOC], F32, tag="ps_ls")
                for kt in range(DT):
                    mm(ps[:],
                       hsT[:, kt, tt_ * P:(tt_ + 1) * P],
                       wsT[:, kt, :],
                       start=(kt == 0), stop=(kt == DT - 1))
                nc.vector.tensor_copy(lsP[:, tt_, :], ps[:])
            for chk in range(NCH):
                ps = ps_b.tile([CLOC, 512], F32, tag="ps_le")
                for kt in range(DT):
                    mm(ps[:],
                       weT[:, kt, :],
                       heT[:, kt, chk * 512:(chk + 1) * 512],
                       start=(kt == 0), stop=(kt == DT - 1))
                nc.vector.tensor_scalar_add(
                    leT[:, chk * 512:(chk + 1) * 512], ps[:], w0_sb[:])
            nc.sync.dma_start(out=leD_h[:], in_=leT[:])

        # ---------------- Phase C: main biaffine loop ----------------
        with (
            tc.tile_pool(name="w1c", bufs=2) as p_w1,
            tc.tile_pool(name="ttp", bufs=2) as p_tt,
            tc.tile_pool(name="lebp", bufs=3) as p_leb,
            tc.tile_pool(name="outp", bufs=6) as p_out,
            tc.tile_pool(name="ps_s1", bufs=4, space="PSUM") as ps_s1,
            tc.tile_pool(name="ps_s2", bufs=4, space="PSUM") as ps_s2,
        ):
            w1c_ap = w1c_h[:].rearrange("c (t p) e -> c p t e", p=P)
            out_ap = out_h[:]

            def stage1(w1t, b):
                tt_t = p_tt.tile([P, DT, 512], DT_S2, tag="tt", name="tt_t")
                for et in range(DT):
                    ps = ps_s1.tile([P, 512], F32, tag="s1", name="ps1")
                    for dt_ in range(DT):
                        mm(ps[:],
                           w1t[:, dt_, et * P:(et + 1) * P],
                           hsT[:, dt_, b * 512:(b + 1) * 512],
                           start=(dt_ == 0), stop=(dt_ == DT - 1))
                    nc.vector.tensor_copy(tt_t[:, et, :], ps[:])
                return tt_t

            def stage2(tt_t, c, b):
                # Le[b,:,c] + w0[c], broadcast across the 128 i-partitions
                leB = p_leb.tile([P, 512], F32, tag="leB", name="leB")
                nc.sync.dma_start(
                    out=leB[:],
                    in_=leD_h[c, b * 512:(b + 1) * 512].partition_broadcast(P))
                for it in range(LT):
                    ps2 = ps_s2.tile([P, 512], F32, tag="s2", name="ps2")
                    for et in range(DT):
                        mm(ps2[:],
                           tt_t[:, et, it * P:(it + 1) * P],
                           heT[:, et, b * 512:(b + 1) * 512],
                           start=(et == 0), stop=(et == DT - 1))
                    o_t = p_out.tile([P, 512], F32, tag="o", name="o_t")
                    # pass 1 (ACT): psum + Ls  -> o_t ; pass 2 (DVE): += Le
                    nc.scalar.activation(
                        o_t[:], ps2[:], Ident,
                        bias=lsP[:, b * LT + it, c:c + 1])
                    nc.vector.tensor_add(o_t[:], o_t[:], leB[:])
                    nc.sync.dma_start(
                        out=out_ap[b, c, it * P:(it + 1) * P, :],
                        in_=o_t[:])

            # Software-pipelined: stage1 of iteration k+1 is emitted before
            # stage2 of iteration k so PE never waits on TT evictions.
            pending = None
            w1t = None
            for c in range(CLOC):
                w1t = p_w1.tile([P, DT, D], DT_S1, tag="w1t", name="w1t")
                nc.gpsimd.dma_start(out=w1t[:], in_=w1c_ap[c])
                for b in range(B):
                    tt_t = stage1(w1t, b)
                    if pending is not None:
                        stage2(*pending)
                    pending = (tt_t, c, b)
            stage2(*pending)
    nc.finalize()
    return nc


def _prep_inputs(inputs):
    """Host-side: transpose/cast/shard. Returns list of 8 in_maps."""
    f32 = np.float32
    np_mlp, np_s1, np_s2 = _np_of(DT_MLP), _np_of(DT_S1), _np_of(DT_S2)

    def cast(x, dt):
        return np.ascontiguousarray(np.asarray(x), dtype=dt)

    h = np.asarray(inputs["hidden_states"], f32)
    xT = cast(h.reshape(T, D).T, np_mlp)
    sw1T = cast(np.asarray(inputs["sw1"], f32).T, np_mlp)
    sw2T = cast(np.asarray(inputs["sw2"], f32).T, np_mlp)
    ew1T = cast(np.asarray(inputs["ew1"], f32).T, np_mlp)
    ew2T = cast(np.asarray(inputs["ew2"], f32).T, np_mlp)
    sb1 = np.ascontiguousarray(inputs["sb1"], f32)
    sb2 = np.ascontiguousarray(inputs["sb2"], f32)
    eb1 = np.ascontiguousarray(inputs["eb1"], f32)
    eb2 = np.ascontiguousarray(inputs["eb2"], f32)
    W1 = np.asarray(inputs["W1"], f32)
    W2w = np.asarray(inputs["W2_w"], f32)
    wsT = np.asarray(W2w[:, :D].T)      # [D, C]
    weT = np.asarray(W2w[:, D:].T)      # [D, C]
    w0 = (np.asarray(inputs["W2_b"], f32)
          + np.asarray(inputs["bias"], f32)).reshape(C, 1)

    in_maps = []
    for m in range(NCORES):
        cs = slice(m * CLOC, (m + 1) * CLOC)
        in_maps.append({
            "xT": xT, "sw1T": sw1T, "sw2T": sw2T,
            "ew1T": ew1T, "ew2T": ew2T,
            "sb1": sb1, "sb2": sb2, "eb1": eb1, "eb2": eb2,
            "w1c": cast(W1[cs], np_s1),
            "wsT": cast(wsT[:, cs], np_s1),
            "weT": cast(weT[:, cs], np_s2),
            "w0": np.ascontiguousarray(w0[cs]),
        })
    return in_maps


def _gather(per_core_outs):
    full = np.concatenate(per_core_outs, axis=1)       # [B, C, L, L]
    return np.ascontiguousarray(full.transpose(0, 2, 3, 1))  # [B, L, L, C]


def kernel(**inputs):
    in_maps = _prep_inputs(inputs)
    nc = build_program()
    res = run_bass_kernel_spmd(nc, in_maps, list(range(NCORES)))
    return _gather([r["out"] for r in res.results])



# revision 6
# speedup vs baseline: 1.2263x; 1.2263x over previous
"""BiaffineSpan TRN2 kernel.

Full-input contract: kernel(**inputs) -> [B, L, L, C] float32.

Sharding: the C=256 bilinear channel dim is split across 8 NeuronCores
(32 channels each).  Each core computes, entirely on-device:

    Hs = MLP_s(hidden)            # [B, L, D]   (dup on every core)
    He = MLP_e(hidden)            # [B, L, D]
    T[c]  = Hs[b] @ W1[c]         # stage 1, per local channel
    S[c]  = T[c] @ He[b].T        # stage 2
    S[c] += Ls[b,i,c] + Le[b,j,c] + W2_b[c] + bias[c]

All matrices are kept transposed ([feature, token]) on-chip so both
matmul stages contract over the partition dim with no on-device
transposes.  Host side only reshapes/casts inputs and concatenates the
8 per-core [B, 32, L, L] outputs.

Everything runs in bf16 operands with fp32 PSUM accumulation (fp8 was
measured at 3.7e-2 rel err vs the 2e-2 gate -- numerically out).  The
schedule aims to keep the PE busy 100% so the HAM clock gate stays at
2.4 GHz:
  - stage-1 PSUM evictions go through the Scalar (ACT) engine so the
    Vector queue (which waits on Le tiles) can never head-of-line-block
    the copies that gate the next matmul group;
  - W1 channel tiles and Le broadcast tiles are prefetched one c ahead;
  - the MLP weight/input tile DMAs are interleaved so the first matmul
    issues after ~1/6 of the bytes land;
  - outputs are written bf16 (upcast on host) to halve DMA.
"""

from contextlib import ExitStack as ExitStack_

import numpy as np
import ml_dtypes

import concourse.bass as bass
import concourse.bacc as bacc
import concourse.mybir as mybir
import concourse.tile as tile
from concourse.bass_utils import run_bass_kernel_spmd

B, L, D, C = 2, 512, 768, 256
NCORES = 8
CLOC = C // NCORES          # 32 channels per core
T = B * L                   # 1024 tokens
P = 128
DT = D // P                 # 6 feature tiles
LT = L // P                 # 4 token tiles per batch el
TT = T // P                 # 8 token tiles total
NCH = T // 512              # 2 moving chunks of 512 tokens

F32 = mybir.dt.float32
BF16 = mybir.dt.bfloat16
MODE = "bf16"


def build_program():
    nc = bacc.Bacc("TRN2", target_bir_lowering=False, debug=False)

    # ---- DRAM parameters (per-core inputs) ----
    xT_h = nc.declare_dram_parameter("xT", [D, T], BF16, isOutput=False)
    sw1T_h = nc.declare_dram_parameter("sw1T", [D, D], BF16, isOutput=False)
    sw2T_h = nc.declare_dram_parameter("sw2T", [D, D], BF16, isOutput=False)
    ew1T_h = nc.declare_dram_parameter("ew1T", [D, D], BF16, isOutput=False)
    ew2T_h = nc.declare_dram_parameter("ew2T", [D, D], BF16, isOutput=False)
    sb1_h = nc.declare_dram_parameter("sb1", [D], F32, isOutput=False)
    sb2_h = nc.declare_dram_parameter("sb2", [D], F32, isOutput=False)
    eb1_h = nc.declare_dram_parameter("eb1", [D], F32, isOutput=False)
    eb2_h = nc.declare_dram_parameter("eb2", [D], F32, isOutput=False)
    w1c_h = nc.declare_dram_parameter("w1c", [CLOC, D, D], BF16, isOutput=False)
    wsT_h = nc.declare_dram_parameter("wsT", [D, CLOC], BF16, isOutput=False)
    weT_h = nc.declare_dram_parameter("weT", [D, CLOC], BF16, isOutput=False)
    w0_h = nc.declare_dram_parameter("w0", [CLOC, 1], F32, isOutput=False)
    out_h = nc.declare_dram_parameter("out", [B, CLOC, L, L], BF16, isOutput=True)
    # DRAM bounce for Le rows, so they can be partition-broadcast back in
    leD_h = nc.dram_tensor("leD", [CLOC, T], BF16)

    Relu = mybir.ActivationFunctionType.Relu
    Ident = mybir.ActivationFunctionType.Identity

    def mm(ps, lhsT, rhs, start, stop):
        nc.tensor.matmul(ps, lhsT, rhs, start=start, stop=stop)

    with tile.TileContext(nc) as tc, ExitStack_() as ctx:
        # persistent pools
        p_h = ctx.enter_context(tc.tile_pool(name="hids", bufs=1))
        p_lin = ctx.enter_context(tc.tile_pool(name="lin", bufs=1))
        p_w1 = ctx.enter_context(tc.tile_pool(name="w1c", bufs=3))
        p_leb = ctx.enter_context(tc.tile_pool(name="lebp", bufs=3))
        p_wse = ctx.enter_context(tc.tile_pool(name="wse", bufs=1))

        h1T = p_h.tile([P, DT, T], BF16, tag="h1")
        hsT = p_h.tile([P, DT, T], BF16, tag="hs")
        heT = p_h.tile([P, DT, T], BF16, tag="he")

        # hoisted small loads: Ws/We/w0 (phase B params)
        wsT = p_wse.tile([P, DT, CLOC], BF16, tag="ws")
        weT = p_wse.tile([P, DT, CLOC], BF16, tag="we")
        w0_sb = p_wse.tile([CLOC, 1], F32, tag="w0")
        nc.sync.dma_start(
            out=wsT[:], in_=wsT_h[:].rearrange("(t p) c -> p t c", p=P))
        nc.sync.dma_start(
            out=weT[:], in_=weT_h[:].rearrange("(t p) c -> p t c", p=P))
        nc.sync.dma_start(out=w0_sb[:], in_=w0_h[:])

        # hoisted prefetch of the first W1 channel tile
        w1c_ap = w1c_h[:].rearrange("c (t p) e -> c p t e", p=P)
        w1_tiles = {}
        w1_tiles[0] = p_w1.tile([P, DT, D], BF16, tag="w1t", name="w1t0")
        nc.scalar.dma_start(out=w1_tiles[0][:], in_=w1c_ap[0])

        # ---------------- Phase A: the two MLPs ----------------
        with (
            tc.tile_pool(name="ph_a", bufs=2) as p_a,
            tc.tile_pool(name="ps_a", bufs=4, space="PSUM") as ps_a,
            tc.tile_pool(name="bias", bufs=1) as p_bias,
        ):
            b_sb = {}
            for nm, h in (("sb1", sb1_h), ("sb2", sb2_h),
                          ("eb1", eb1_h), ("eb2", eb2_h)):
                b_sb[nm] = p_bias.tile([P, DT], F32, tag=nm, name=nm)
                nc.sync.dma_start(
                    out=b_sb[nm][:], in_=h[:].rearrange("(t p) -> p t", p=P)
                )
            # interleave x-tile and first-weight-tile loads so the first
            # matmul only waits for the kt=0 pair
            xT = p_a.tile([P, DT, T], BF16, tag="x")
            xT_ap = xT_h[:].rearrange("(t p) n -> t p n", p=P)
            w_first = p_a.tile([P, DT, D], BF16, tag="w_mlp")
            w_first_ap = sw1T_h[:].rearrange("(t p) o -> t p o", p=P)
            for kt in range(DT):
                nc.gpsimd.dma_start(out=xT[:, kt, :], in_=xT_ap[kt])
                nc.gpsimd.dma_start(out=w_first[:, kt, :], in_=w_first_ap[kt])

            def linear(inT, w_h, bias_t, outT, relu, wT=None):
                if wT is None:
                    wT = p_a.tile([P, DT, D], BF16, tag="w_mlp")
                    w_ap = w_h[:].rearrange("(t p) o -> t p o", p=P)
                    for kt in range(DT):
                        nc.gpsimd.dma_start(out=wT[:, kt, :], in_=w_ap[kt])
                for ot in range(DT):
                    for chk in range(NCH):
                        ps = ps_a.tile([P, 512], F32, tag="ps_mlp")
                        for kt in range(DT):
                            mm(ps[:],
                               wT[:, kt, ot * P:(ot + 1) * P],
                               inT[:, kt, chk * 512:(chk + 1) * 512],
                               start=(kt == 0), stop=(kt == DT - 1))
                        nc.scalar.activation(
                            outT[:, ot, chk * 512:(chk + 1) * 512], ps[:],
                            Relu if relu else Ident,
                            bias=bias_t[:, ot:ot + 1])

            linear(xT, None, b_sb["sb1"], h1T, relu=True, wT=w_first)
            linear(h1T, sw2T_h, b_sb["sb2"], hsT, relu=False)
            linear(xT, ew1T_h, b_sb["eb1"], h1T, relu=True)
            linear(h1T, ew2T_h, b_sb["eb2"], heT, relu=False)

        # ---------------- Phase B: linear terms ----------------
        # LsP [tok_tile, 128, CLOC]  (token on partitions, channel free)
        # LeT [CLOC, T] (+ w0)      (channel on partitions, token free)
        lsP = p_lin.tile([P, TT, CLOC], F32, tag="lsP")
        leT = p_lin.tile([CLOC, T], BF16, tag="leT")

        with tc.tile_pool(name="ps_b", bufs=2, space="PSUM") as ps_b:
            for tt_ in range(TT):
                ps = ps_b.tile([P, CLOC], F32, tag="ps_ls")
                for kt in range(DT):
                    mm(ps[:],
                       hsT[:, kt, tt_ * P:(tt_ + 1) * P],
                       wsT[:, kt, :],
                       start=(kt == 0), stop=(kt == DT - 1))
                nc.vector.tensor_copy(lsP[:, tt_, :], ps[:])
            for chk in range(NCH):
                ps = ps_b.tile([CLOC, 512], F32, tag="ps_le")
                for kt in range(DT):
                    mm(ps[:],
                       weT[:, kt, :],
                       heT[:, kt, chk * 512:(chk + 1) * 512],
                       start=(kt == 0), stop=(kt == DT - 1))
                nc.vector.tensor_scalar_add(
                    leT[:, chk * 512:(chk + 1) * 512], ps[:], w0_sb[:])
            nc.sync.dma_start(out=leD_h[:], in_=leT[:])

        # ---------------- Phase C: main biaffine loop ----------------
        with (
            tc.tile_pool(name="ttp", bufs=2) as p_tt,
            tc.tile_pool(name="outp", bufs=6) as p_out,
            tc.tile_pool(name="ps_s1", bufs=4, space="PSUM") as ps_s1,
            tc.tile_pool(name="ps_s2", bufs=4, space="PSUM") as ps_s2,
        ):
            out_ap = out_h[:]

            # Le[c, :] + w0[c] broadcast tiles, both b halves: [128, 2, 512]
            leb_tiles = {}

            def prefetch_leb(c):
                t_ = p_leb.tile([P, NCH, 512], BF16, tag="leB", name="leB")
                nc.sync.dma_start(
                    out=t_[:], in_=leD_h[c, :].partition_broadcast(P)
                    .rearrange("p (n f) -> p n f", f=512))
                leb_tiles[c] = t_

            prefetch_leb(0)

            def stage1(w1t, b):
                tt_t = p_tt.tile([P, DT, 512], BF16, tag="tt", name="tt_t")
                for et in range(DT):
                    ps = ps_s1.tile([P, 512], F32, tag="s1", name="ps1")
                    for dt_ in range(DT):
                        mm(ps[:],
                           w1t[:, dt_, et * P:(et + 1) * P],
                           hsT[:, dt_, b * 512:(b + 1) * 512],
                           start=(dt_ == 0), stop=(dt_ == DT - 1))
                    # ACT engine eviction: never queued behind Le-waiting ops
                    nc.scalar.activation(tt_t[:, et, :], ps[:], Ident)
                return tt_t

            def stage2(tt_t, c, b):
                leB = leb_tiles[c]
                for it in range(LT):
                    ps2 = ps_s2.tile([P, 512], F32, tag="s2", name="ps2")
                    for et in range(DT):
                        mm(ps2[:],
                           tt_t[:, et, it * P:(it + 1) * P],
                           heT[:, et, b * 512:(b + 1) * 512],
                           start=(et == 0), stop=(et == DT - 1))
                    o_t = p_out.tile([P, 512], BF16, tag="o", name="o_t")
                    # pass 1 (ACT): psum + Ls  -> o_t ; pass 2 (DVE): += Le
                    nc.scalar.activation(
                        o_t[:], ps2[:], Ident,
                        bias=lsP[:, b * LT + it, c:c + 1])
                    nc.vector.tensor_add(o_t[:], o_t[:], leB[:, b, :])
                    nc.sync.dma_start(
                        out=out_ap[b, c, it * P:(it + 1) * P, :],
                        in_=o_t[:])

            # Software-pipelined: stage1 of iteration k+1 is emitted before
            # stage2 of iteration k so PE never waits on TT evictions.
            pending = None
            for c in range(CLOC):
                if c + 1 < CLOC:
                    w1_tiles[c + 1] = p_w1.tile(
                        [P, DT, D], BF16, tag="w1t", name="w1t")
                    nc.gpsimd.dma_start(
                        out=w1_tiles[c + 1][:], in_=w1c_ap[c + 1])
                    prefetch_leb(c + 1)
                for b in range(B):
                    tt_t = stage1(w1_tiles[c], b)
                    if pending is not None:
                        stage2(*pending)
                    pending = (tt_t, c, b)
                w1_tiles.pop(c - 1, None)
                leb_tiles.pop(c - 1, None)
            stage2(*pending)
    nc.finalize()
    return nc


def _prep_inputs(inputs):
    """Host-side: transpose/cast/shard. Returns list of 8 in_maps."""
    f32 = np.float32
    bf = ml_dtypes.bfloat16

    def cast(x, dt=bf):
        return np.ascontiguousarray(np.asarray(x, f32), dtype=dt)

    h = np.asarray(inputs["hidden_states"], f32)
    xT = cast(h.reshape(T, D).T)
    sw1T = cast(np.asarray(inputs["sw1"], f32).T)
    sw2T = cast(np.asarray(inputs["sw2"], f32).T)
    ew1T = cast(np.asarray(inputs["ew1"], f32).T)
    ew2T = cast(np.asarray(inputs["ew2"], f32).T)
    sb1 = np.ascontiguousarray(inputs["sb1"], f32)
    sb2 = np.ascontiguousarray(inputs["sb2"], f32)
    eb1 = np.ascontiguousarray(inputs["eb1"], f32)
    eb2 = np.ascontiguousarray(inputs["eb2"], f32)
    W1 = np.asarray(inputs["W1"], f32)
    W2w = np.asarray(inputs["W2_w"], f32)
    wsT = np.asarray(W2w[:, :D].T)      # [D, C]
    weT = np.asarray(W2w[:, D:].T)      # [D, C]
    w0 = (np.asarray(inputs["W2_b"], f32)
          + np.asarray(inputs["bias"], f32)).reshape(C, 1)

    in_maps = []
    for m in range(NCORES):
        cs = slice(m * CLOC, (m + 1) * CLOC)
        in_maps.append({
            "xT": xT, "sw1T": sw1T, "sw2T": sw2T,
            "ew1T": ew1T, "ew2T": ew2T,
            "sb1": sb1, "sb2": sb2, "eb1": eb1, "eb2": eb2,
            "w1c": cast(W1[cs]),
            "wsT": cast(wsT[:, cs]),
            "weT": cast(weT[:, cs]),
            "w0": np.ascontiguousarray(w0[cs]),
        })
    return in_maps


def _gather(per_core_outs):
    full = np.concatenate(
        [np.asarray(o, np.float32) for o in per_core_outs], axis=1)
    return np.ascontiguousarray(full.transpose(0, 2, 3, 1))  # [B, L, L, C]


def kernel(**inputs):
    in_maps = _prep_inputs(inputs)
    nc = build_program()
    res = run_bass_kernel_spmd(nc, in_maps, list(range(NCORES)))
    return _gather([r["out"] for r in res.results])
